# revision 2
# baseline (speedup 1.0000x reference)
"""Causal self-attention (RoPE, B=4 S=2048 D=2048 H=16) on 8 Trainium2 cores, v4.

Sharding: core c = 2*b + hh -> batch b, head-half hh (8 heads/core).
Host sums the two output-projection partials per batch.

v4 design (vs v3's split-fp8 scheme): PE passes stream 512 px at 1 px/cycle
regardless of dtype, so the 3-term fp8-DR projection (24 passes/tile) is
strictly worse than plain 16-bit (16 passes/tile).  Everything runs fp16
(8x the mantissa of bf16, same speed):
- Projections: fp16 x (moving) @ fp16 W (stationary), f32 PSUM.
- QK^T / AV / denominator: fp16.
- Softmax: exp on Act (scale folds 1/sqrt(hd)), fp16 probs; causal mask
  multiplied on diagonal blocks only; denominator via ones-matmul;
  1/den via DVE reciprocal_approx_fast (~5x cheaper than reciprocal).
- Software-pipelined head loop: per iteration h the PE runs
  proj(h) -> transposes(h) -> dps/yps(h-1) -> QK^T(h), so the Act engine's
  exp(h) backlog drains during proj(h+1) and never stalls the PE.
"""
import sys

try:
    import concourse.bass as _chk  # noqa: F401
except ImportError:
    for p in ("/opt/trn_rl_repo", "/root/.axon_site/_ro/trn_rl_repo"):
        if p not in sys.path:
            sys.path.insert(0, p)

import math
import numpy as np

import concourse.bass as bass
import concourse.tile as tile
from concourse import mybir
from concourse.bass_utils import run_bass_kernel_spmd

N_CORES = 8
B = 4
D = 2048
H = 16
HD = 128
HPC = 8
ROPE_BASE = 10000.0
F32 = mybir.dt.float32
F16 = mybir.dt.float16
EXP = mybir.ActivationFunctionType.Exp
SCALE = 1.0 / math.sqrt(HD)


def split_ctrl_waits(nc, maxw=1):
    """Walrus in this env can't encode >1 sem-wait on many instruction
    formats; move extras onto preceding same-engine NoOps."""
    nid = [0]
    for f in nc.m.functions:
        for b in f.blocks:
            new_insts = []
            for inst in b.instructions:
                si = inst.sync_info
                if si is not None and si.on_wait is not None and len(si.on_wait) > maxw:
                    waits = list(si.on_wait)
                    while len(waits) > maxw:
                        chunk, waits = waits[:maxw], waits[maxw:]
                        nid[0] += 1
                        nop = mybir.InstNoOp(
                            name=f"I-waitsplit-{nid[0]}",
                            ins=[], outs=[],
                            sync_info=mybir.SyncInfo(on_wait=chunk, on_update=[]),
                        )
                        nop.engine = inst.engine
                        new_insts.append(nop)
                    si.on_wait = waits
                new_insts.append(inst)
            b.instructions[:] = new_insts


def build_nc(S=2048, repeat=1):
    KT = D // 128
    NQ = S // 512
    NK = S // 128

    nc = bass.Bass("TRN2", debug=False, num_devices=N_CORES)

    xd = nc.dram_tensor("xt", [KT, 128, S], F16, kind="ExternalInput")
    w_d = {}
    for nm in ("wq", "wk", "wv"):
        w_d[nm] = nc.dram_tensor(nm, [HPC, 128, KT, 128], F16, kind="ExternalInput")
    wod = nc.dram_tensor("wo", [HPC, 128, D], F16, kind="ExternalInput")
    cosf = nc.dram_tensor("cosf", [128, S], F16, kind="ExternalInput")
    sinf = nc.dram_tensor("sinf", [128, S], F16, kind="ExternalInput")
    dmask_d = nc.dram_tensor("dmask", [128, 4 * 512], F16, kind="ExternalInput")
    ones_d = nc.dram_tensor("ones", [128, 128], F16, kind="ExternalInput")
    ident_d = nc.dram_tensor("ident", [128, 128], F16, kind="ExternalInput")
    out = nc.dram_tensor("out", [S, D], F32, kind="ExternalOutput")

    with tile.TileContext(nc) as tc:
        with tc.tile_pool(name="const", bufs=1) as cp:
            xt = [cp.tile([128, KT, 512], F16, name=f"xt{cx}") for cx in range(NQ)]
            cos_t = cp.tile([128, S], F16, name="cos_t")
            sin_t = cp.tile([128, S], F16, name="sin_t")
            dmask = cp.tile([128, 4 * 512], F16, name="dmask_t")
            ones2 = cp.tile([128, 128], F16, name="ones2")
            ident = cp.tile([128, 128], F16, name="ident_t")
            y_sb = cp.tile([128, HPC, S], F16, name="y_sb")

            for _rep in range(repeat):
                for cx in range(NQ):
                    for k in range(KT):
                        nc.sync.dma_start(xt[cx][:, k],
                                          xd[k][:, cx * 512:(cx + 1) * 512])
                nc.sync.dma_start(cos_t[:], cosf[:])
                nc.sync.dma_start(sin_t[:], sinf[:])
                nc.sync.dma_start(dmask[:], dmask_d[:])
                nc.sync.dma_start(ones2[:], ones_d[:])
                nc.sync.dma_start(ident[:], ident_d[:])
                _body(nc, tc, S, KT, NQ, NK, xt, w_d, wod,
                      cos_t, sin_t, dmask, ones2, ident, y_sb, out)

    split_ctrl_waits(nc)
    return nc


def _body(nc, tc, S, KT, NQ, NK, xt, w_d, wod,
          cos_t, sin_t, dmask, ones2, ident, y_sb, out):
    with tc.tile_pool(name="wst", bufs=2) as wst, \
         tc.tile_pool(name="qkp", bufs=1) as qkp, \
         tc.tile_pool(name="vtp", bufs=1) as vtp, \
         tc.tile_pool(name="swp", bufs=1) as swp, \
         tc.tile_pool(name="hb", bufs=2) as hb, \
         tc.tile_pool(name="ptb", bufs=1) as ptb, \
         tc.tile_pool(name="rcp", bufs=1) as rcp, \
         tc.tile_pool(name="pp", bufs=2, space="PSUM") as pp, \
         tc.tile_pool(name="ptr", bufs=1, space="PSUM") as ptr, \
         tc.tile_pool(name="pss", bufs=3, space="PSUM") as pss, \
         tc.tile_pool(name="psy", bufs=1, space="PSUM") as psy, \
         tc.tile_pool(name="psd", bufs=1, space="PSUM") as psd:

        prev = None  # (h, pts list, v8 tile)

        def dy_phase(ph, pts, v8):
            # denominator + AV for head ph (exps long finished).  The
            # reciprocal (3.4us on DVE) runs concurrently with the yps
            # matmuls: dps -> sbuf copy frees the PSUM bank immediately.
            for qg in range(NQ):
                nkt = 4 * qg + 4
                dps = psd.tile([128, 512], F32, name=f"dps{ph}_{qg}", tag="dps")
                yps = psy.tile([128, 512], F32, name=f"yps{ph}_{qg}", tag="yps")
                for kt in range(nkt):
                    nc.tensor.matmul(dps[:], ones2[:], pts[qg][kt][:],
                                     start=(kt == 0), stop=(kt == nkt - 1))
                dsb = rcp.tile([128, 512], F32, name=f"dsb{ph}_{qg}", tag="dsb")
                rec = rcp.tile([128, 512], F32, name=f"rec{ph}_{qg}", tag="rec")
                nc.vector.tensor_copy(dsb[:], dps[:])
                nc.vector.reciprocal(rec[:], dsb[:])
                for kt in range(nkt):
                    nc.tensor.matmul(yps[:], v8[:, kt], pts[qg][kt][:],
                                     start=(kt == 0), stop=(kt == nkt - 1))
                sl = slice(qg * 512, (qg + 1) * 512)
                nc.vector.tensor_mul(y_sb[:, ph, sl], yps[:], rec[:])

        for h in range(HPC):
            # ---- stream this head's weights ----
            w = {}
            for nm in ("wq", "wk", "wv"):
                t = wst.tile([128, KT, 128], F16, name=f"{nm}{h}", tag=nm)
                nc.sync.dma_start(t[:], w_d[nm][h])
                w[nm] = t

            # ---- projections: plain fp16, 16 passes per [128,512] tile ----
            qraw = qkp.tile([128, S], F16, name=f"qraw{h}", tag="qraw")
            kraw = qkp.tile([128, S], F16, name=f"kraw{h}", tag="kraw")
            vT = vtp.tile([128, S], F16, name=f"vT{h}", tag="vT")
            for dst, wt in ((qraw, w["wq"]), (kraw, w["wk"]), (vT, w["wv"])):
                for cx in range(NQ):
                    po = pp.tile([128, 512], F32, name=f"po{h}_{cx}", tag="proj")
                    for k in range(KT):
                        nc.tensor.matmul(po[:], wt[:, k], xt[cx][:, k],
                                         start=(k == 0), stop=(k == KT - 1))
                    nc.scalar.copy(dst[:, cx * 512:(cx + 1) * 512], po[:])

            # ---- v: transpose to [key, hd] layout ----
            v8 = hb.tile([128, NK, 128], F16, name=f"v8_{h}", tag="v8")
            for t in range(NK):
                tp = ptr.tile([128, 128], F16, name=f"tp{h}_{t}", tag="tr")
                nc.tensor.transpose(tp[:], vT[:, t * 128:(t + 1) * 128], ident[:])
                nc.vector.tensor_copy(v8[:, t], tp[:])

            # ---- RoPE in place; swap-halves copies on Act ----
            sw = swp.tile([128, S], F16, name=f"sw{h}", tag="sw")
            for src_t in (qraw, kraw):
                nc.scalar.copy(sw[0:64, :], src_t[64:128, :])
                nc.scalar.copy(sw[64:128, :], src_t[0:64, :])
                nc.vector.tensor_mul(src_t[:], src_t[:], cos_t[:])
                nc.vector.tensor_mul(sw[:], sw[:], sin_t[:])
                nc.vector.tensor_add(src_t[:], src_t[:], sw[:])

            # ---- dps/yps for the previous head (its exps are done) ----
            if prev is not None:
                dy_phase(*prev)

            # ---- QK^T + exp for this head ----
            pts = []
            for qg in range(NQ):
                nkt = 4 * qg + 4
                row = []
                for kt in range(nkt):
                    sps = pss.tile([128, 512], F32, name=f"sps{h}_{qg}_{kt}",
                                   tag="sps")
                    nc.tensor.matmul(sps[:], kraw[:, kt * 128:(kt + 1) * 128],
                                     qraw[:, qg * 512:(qg + 1) * 512],
                                     start=True, stop=True)
                    pt = ptb.tile([128, 512], F16, name=f"pt{h}_{qg}_{kt}",
                                  tag=f"pt{qg}_{kt}")
                    nc.scalar.activation(pt[:], sps[:], EXP, scale=SCALE)
                    jj = kt - 4 * qg
                    if jj >= 0:
                        nc.vector.tensor_mul(pt[:], pt[:],
                                             dmask[:, jj * 512:(jj + 1) * 512])
                    row.append(pt)
                pts.append(row)
            prev = (h, pts, v8)

        dy_phase(*prev)

    # ---- output projection: plain fp16, 8 passes per [128,512] tile ----
    with tc.tile_pool(name="wop", bufs=1) as wop, \
         tc.tile_pool(name="cop", bufs=2) as cop, \
         tc.tile_pool(name="cps", bufs=2, space="PSUM") as cps:
        wos = wop.tile([128, HPC, D], F16, name="wos")
        for hh in range(HPC):
            nc.sync.dma_start(wos[:, hh], wod[hh])
        for st in range(NK):
            pos = [cps.tile([128, 512], F32, name=f"cpo{st}_{i}", tag=f"cpo{i}")
                   for i in range(4)]
            for oc in range(4):
                for hp in range(HPC):
                    nc.tensor.matmul(
                        pos[oc][:],
                        y_sb[:, hp, st * 128:(st + 1) * 128],
                        wos[:, hp, oc * 512:(oc + 1) * 512],
                        start=(hp == 0), stop=(hp == HPC - 1))
            for oc in range(4):
                ot = cop.tile([128, 512], F32, name=f"cot{st}_{oc}", tag="cot")
                nc.scalar.copy(ot[:], pos[oc][:])
                nc.sync.dma_start(out[st * 128:(st + 1) * 128,
                                      oc * 512:(oc + 1) * 512], ot[:])


def prep_in_maps(x, positions, Wqkv, Wout, S=2048):
    KT = D // 128
    QF = HPC * HD

    inv_freq = 1.0 / (ROPE_BASE ** (np.arange(0, HD, 2, dtype=np.float64) / HD))
    pos = np.asarray(positions).astype(np.float64)[:S]
    freq = pos[None, :] * inv_freq[:, None]
    c = np.cos(freq).astype(np.float32)
    s = np.sin(freq).astype(np.float32)
    cosf = np.vstack([c, c]).astype(np.float16)
    sinf = np.vstack([-s, s]).astype(np.float16)

    dm = np.zeros((128, 4, 512), np.float32)
    for j in range(4):
        dm[:, j, 128 * j:128 * (j + 1)] = np.triu(np.ones((128, 128), np.float32))
        dm[:, j, 128 * (j + 1):] = 1.0
    dmask = dm.reshape(128, 4 * 512).astype(np.float16)

    ones8 = np.ones((128, 128), np.float16)
    ident = np.eye(128, dtype=np.float16)

    perm = np.concatenate([np.arange(0, HD, 2), np.arange(1, HD, 2)])

    in_maps = []
    for c_id in range(N_CORES):
        b, hh = c_id // 2, c_id % 2
        xT = np.ascontiguousarray(x[b, :S, :].T).astype(np.float16)  # [D, S]
        f0 = hh * QF
        Wq = Wqkv[:, f0:f0 + QF]
        Wk = Wqkv[:, D + f0:D + f0 + QF]
        Wv = Wqkv[:, 2 * D + f0:2 * D + f0 + QF]
        Wqp = Wq.reshape(D, HPC, HD)[:, :, perm]       # [D, h, o]
        Wkp = Wk.reshape(D, HPC, HD)[:, :, perm]
        Wvp = Wv.reshape(D, HPC, HD)

        def pack(Wn):
            # [D, h, o] -> [h][d%128][k][o]
            return np.ascontiguousarray(
                Wn.reshape(KT, 128, HPC, HD).transpose(2, 1, 0, 3)
            ).astype(np.float16)

        wo = np.ascontiguousarray(
            Wout[f0:f0 + QF, :].reshape(HPC, 128, D)).astype(np.float16)
        in_maps.append({
            "xt": xT.reshape(KT, 128, S),
            "wq": pack(Wqp), "wk": pack(Wkp), "wv": pack(Wvp),
            "wo": wo,
            "cosf": cosf, "sinf": sinf, "dmask": dmask, "ones": ones8,
            "ident": ident,
        })
    return in_maps


def kernel(x, positions, mask, Wqkv, Wout):
    x = np.asarray(x, dtype=np.float32)
    Wqkv = np.asarray(Wqkv, dtype=np.float32)
    Wout = np.asarray(Wout, dtype=np.float32)
    S = x.shape[1]
    nc = build_nc(S=S)
    in_maps = prep_in_maps(x, positions, Wqkv, Wout, S=S)
    res = run_bass_kernel_spmd(nc, in_maps, core_ids=list(range(N_CORES)))
    outs = [res.results[c]["out"] for c in range(N_CORES)]
    full = np.stack([outs[2 * b] + outs[2 * b + 1] for b in range(B)], axis=0)
    return full.astype(np.float32)


# revision 3
# speedup vs baseline: 1.1770x; 1.1770x over previous
"""Causal self-attention (RoPE, B=4 S=2048 D=2048 H=16) on 8 Trainium2 cores, v4.

Sharding: core c = 2*b + hh -> batch b, head-half hh (8 heads/core).
Host sums the two output-projection partials per batch.

v4 design (vs v3's split-fp8 scheme): PE passes stream 512 px at 1 px/cycle
regardless of dtype, so the 3-term fp8-DR projection (24 passes/tile) is
strictly worse than plain 16-bit (16 passes/tile).  Everything runs fp16
(8x the mantissa of bf16, same speed):
- Projections: fp16 x (moving) @ fp16 W (stationary), f32 PSUM.
- QK^T / AV / denominator: fp16.
- Softmax: exp on Act (scale folds 1/sqrt(hd)), fp16 probs; causal mask
  multiplied on diagonal blocks only; denominator via ones-matmul;
  1/den via DVE reciprocal_approx_fast (~5x cheaper than reciprocal).
- Software-pipelined head loop: per iteration h the PE runs
  proj(h) -> transposes(h) -> dps/yps(h-1) -> QK^T(h), so the Act engine's
  exp(h) backlog drains during proj(h+1) and never stalls the PE.
"""
import sys

try:
    import concourse.bass as _chk  # noqa: F401
except ImportError:
    for p in ("/opt/trn_rl_repo", "/root/.axon_site/_ro/trn_rl_repo"):
        if p not in sys.path:
            sys.path.insert(0, p)

import math
import numpy as np

import concourse.bass as bass
import concourse.tile as tile
from concourse import mybir
from concourse.bass_utils import run_bass_kernel_spmd

N_CORES = 8
B = 4
D = 2048
H = 16
HD = 128
HPC = 8
ROPE_BASE = 10000.0
F32 = mybir.dt.float32
F16 = mybir.dt.float16
EXP = mybir.ActivationFunctionType.Exp
SCALE = 1.0 / math.sqrt(HD)


def split_ctrl_waits(nc, maxw=1):
    """Walrus in this env can't encode >1 sem-wait on many instruction
    formats; move extras onto preceding same-engine NoOps."""
    nid = [0]
    for f in nc.m.functions:
        for b in f.blocks:
            new_insts = []
            for inst in b.instructions:
                si = inst.sync_info
                if si is not None and si.on_wait is not None and len(si.on_wait) > maxw:
                    waits = list(si.on_wait)
                    while len(waits) > maxw:
                        chunk, waits = waits[:maxw], waits[maxw:]
                        nid[0] += 1
                        nop = mybir.InstNoOp(
                            name=f"I-waitsplit-{nid[0]}",
                            ins=[], outs=[],
                            sync_info=mybir.SyncInfo(on_wait=chunk, on_update=[]),
                        )
                        nop.engine = inst.engine
                        new_insts.append(nop)
                    si.on_wait = waits
                new_insts.append(inst)
            b.instructions[:] = new_insts


def build_nc(S=2048, repeat=1):
    KT = D // 128
    NQ = S // 512
    NK = S // 128

    nc = bass.Bass("TRN2", debug=False, num_devices=N_CORES)

    xd = nc.dram_tensor("xt", [KT, 128, S], F16, kind="ExternalInput")
    w_d = {}
    for nm in ("wq", "wk", "wv"):
        w_d[nm] = nc.dram_tensor(nm, [HPC, 128, KT, 128], F16, kind="ExternalInput")
    wod = nc.dram_tensor("wo", [HPC, 128, D], F16, kind="ExternalInput")
    cosf = nc.dram_tensor("cosf", [128, S], F16, kind="ExternalInput")
    sinf = nc.dram_tensor("sinf", [128, S], F16, kind="ExternalInput")
    dmask_d = nc.dram_tensor("dmask", [128, 4 * 512], F16, kind="ExternalInput")
    ones_d = nc.dram_tensor("ones", [128, 128], F16, kind="ExternalInput")
    ident_d = nc.dram_tensor("ident", [128, 128], F16, kind="ExternalInput")
    out = nc.dram_tensor("out", [S, D], F32, kind="ExternalOutput")

    with tile.TileContext(nc) as tc:
        with tc.tile_pool(name="const", bufs=1) as cp:
            xt = [cp.tile([128, KT, 512], F16, name=f"xt{cx}") for cx in range(NQ)]
            cos_t = cp.tile([128, S], F16, name="cos_t")
            sin_t = cp.tile([128, S], F16, name="sin_t")
            dmask = cp.tile([128, 4 * 512], F16, name="dmask_t")
            ones2 = cp.tile([128, 128], F16, name="ones2")
            ident = cp.tile([128, 128], F16, name="ident_t")
            y_sb = cp.tile([128, HPC, S], F16, name="y_sb")

            for _rep in range(repeat):
                for cx in range(NQ):
                    for k in range(KT):
                        nc.sync.dma_start(xt[cx][:, k],
                                          xd[k][:, cx * 512:(cx + 1) * 512])
                nc.sync.dma_start(cos_t[:], cosf[:])
                nc.sync.dma_start(sin_t[:], sinf[:])
                nc.sync.dma_start(dmask[:], dmask_d[:])
                nc.sync.dma_start(ones2[:], ones_d[:])
                nc.sync.dma_start(ident[:], ident_d[:])
                _body(nc, tc, S, KT, NQ, NK, xt, w_d, wod,
                      cos_t, sin_t, dmask, ones2, ident, y_sb, out)

    split_ctrl_waits(nc)
    return nc


def _body(nc, tc, S, KT, NQ, NK, xt, w_d, wod,
          cos_t, sin_t, dmask, ones2, ident, y_sb, out):
    with tc.tile_pool(name="wst", bufs=2) as wst, \
         tc.tile_pool(name="qkp", bufs=1) as qkp, \
         tc.tile_pool(name="vtp", bufs=1) as vtp, \
         tc.tile_pool(name="swp", bufs=1) as swp, \
         tc.tile_pool(name="hb", bufs=2) as hb, \
         tc.tile_pool(name="ptb", bufs=1) as ptb, \
         tc.tile_pool(name="rcp", bufs=1) as rcp, \
         tc.tile_pool(name="pp", bufs=2, space="PSUM") as pp, \
         tc.tile_pool(name="ptr", bufs=1, space="PSUM") as ptr, \
         tc.tile_pool(name="pss", bufs=3, space="PSUM") as pss, \
         tc.tile_pool(name="psy", bufs=1, space="PSUM") as psy, \
         tc.tile_pool(name="psd", bufs=1, space="PSUM") as psd:

        prev = None  # (h, pts list, v8 tile)

        def dy_phase(ph, pts, v8):
            # denominator + AV for head ph (exps long finished).  The
            # reciprocal (3.4us on DVE) runs concurrently with the yps
            # matmuls: dps -> sbuf copy frees the PSUM bank immediately.
            for qg in range(NQ):
                nkt = 4 * qg + 4
                dps = psd.tile([128, 512], F32, name=f"dps{ph}_{qg}", tag="dps")
                yps = psy.tile([128, 512], F32, name=f"yps{ph}_{qg}", tag="yps")
                for kt in range(nkt):
                    nc.tensor.matmul(dps[:], ones2[:], pts[qg][kt][:],
                                     start=(kt == 0), stop=(kt == nkt - 1))
                dsb = rcp.tile([128, 512], F32, name=f"dsb{ph}_{qg}", tag="dsb")
                rec = rcp.tile([128, 512], F32, name=f"rec{ph}_{qg}", tag="rec")
                nc.vector.tensor_copy(dsb[:], dps[:])
                nc.vector.reciprocal(rec[:], dsb[:])
                for kt in range(nkt):
                    nc.tensor.matmul(yps[:], v8[:, kt], pts[qg][kt][:],
                                     start=(kt == 0), stop=(kt == nkt - 1))
                sl = slice(qg * 512, (qg + 1) * 512)
                nc.vector.tensor_mul(y_sb[:, ph, sl], yps[:], rec[:])

        for h in range(HPC):
            # ---- stream this head's weights ----
            w = {}
            for nm in ("wq", "wk", "wv"):
                t = wst.tile([128, KT, 128], F16, name=f"{nm}{h}", tag=nm)
                nc.sync.dma_start(t[:], w_d[nm][h])
                w[nm] = t

            # ---- projections: plain fp16, 16 passes per [128,512] tile ----
            qraw = qkp.tile([128, S], F16, name=f"qraw{h}", tag="qraw")
            kraw = qkp.tile([128, S], F16, name=f"kraw{h}", tag="kraw")
            vT = vtp.tile([128, S], F16, name=f"vT{h}", tag="vT")
            for dst, wt in ((qraw, w["wq"]), (kraw, w["wk"]), (vT, w["wv"])):
                for cx in range(NQ):
                    po = pp.tile([128, 512], F32, name=f"po{h}_{cx}", tag="proj")
                    for k in range(KT):
                        nc.tensor.matmul(po[:], wt[:, k], xt[cx][:, k],
                                         start=(k == 0), stop=(k == KT - 1))
                    nc.scalar.copy(dst[:, cx * 512:(cx + 1) * 512], po[:])

            # ---- v: transpose to [key, hd] layout, 4 per psum tile ----
            v8 = hb.tile([128, NK, 128], F16, name=f"v8_{h}", tag="v8")
            for g in range(NK // 4):
                tpq = ptr.tile([128, 512], F16, name=f"tp{h}_{g}", tag="tr")
                for i in range(4):
                    t = 4 * g + i
                    nc.tensor.transpose(tpq[:, i * 128:(i + 1) * 128],
                                        vT[:, t * 128:(t + 1) * 128], ident[:])
                nc.vector.tensor_copy(v8[:, 4 * g:4 * g + 4], tpq[:])

            # ---- RoPE in place; swap-halves copies on Act ----
            sw = swp.tile([128, S], F16, name=f"sw{h}", tag="sw")
            for src_t in (qraw, kraw):
                nc.scalar.copy(sw[0:64, :], src_t[64:128, :])
                nc.scalar.copy(sw[64:128, :], src_t[0:64, :])
                nc.vector.tensor_mul(src_t[:], src_t[:], cos_t[:])
                nc.vector.tensor_mul(sw[:], sw[:], sin_t[:])
                nc.vector.tensor_add(src_t[:], src_t[:], sw[:])

            # ---- dps/yps for the previous head (its exps are done) ----
            if prev is not None:
                dy_phase(*prev)

            # ---- QK^T + exp for this head ----
            pts = []
            for qg in range(NQ):
                nkt = 4 * qg + 4
                row = []
                for kt in range(nkt):
                    sps = pss.tile([128, 512], F32, name=f"sps{h}_{qg}_{kt}",
                                   tag="sps")
                    nc.tensor.matmul(sps[:], kraw[:, kt * 128:(kt + 1) * 128],
                                     qraw[:, qg * 512:(qg + 1) * 512],
                                     start=True, stop=True)
                    pt = ptb.tile([128, 512], F16, name=f"pt{h}_{qg}_{kt}",
                                  tag=f"pt{qg}_{kt}")
                    nc.scalar.activation(pt[:], sps[:], EXP, scale=SCALE)
                    jj = kt - 4 * qg
                    if jj >= 0:
                        nc.vector.tensor_mul(pt[:], pt[:],
                                             dmask[:, jj * 512:(jj + 1) * 512])
                    row.append(pt)
                pts.append(row)
            prev = (h, pts, v8)

        dy_phase(*prev)

    # ---- output projection: plain fp16, 8 passes per [128,512] tile ----
    with tc.tile_pool(name="wop", bufs=1) as wop, \
         tc.tile_pool(name="cop", bufs=2) as cop, \
         tc.tile_pool(name="cps", bufs=2, space="PSUM") as cps:
        wos = wop.tile([128, HPC, D], F16, name="wos")
        for hh in range(HPC):
            nc.sync.dma_start(wos[:, hh], wod[hh])
        for st in range(NK):
            pos = [cps.tile([128, 512], F32, name=f"cpo{st}_{i}", tag=f"cpo{i}")
                   for i in range(4)]
            for oc in range(4):
                for hp in range(HPC):
                    nc.tensor.matmul(
                        pos[oc][:],
                        y_sb[:, hp, st * 128:(st + 1) * 128],
                        wos[:, hp, oc * 512:(oc + 1) * 512],
                        start=(hp == 0), stop=(hp == HPC - 1))
            for oc in range(4):
                ot = cop.tile([128, 512], F32, name=f"cot{st}_{oc}", tag="cot")
                nc.scalar.copy(ot[:], pos[oc][:])
                nc.sync.dma_start(out[st * 128:(st + 1) * 128,
                                      oc * 512:(oc + 1) * 512], ot[:])


def prep_in_maps(x, positions, Wqkv, Wout, S=2048):
    KT = D // 128
    QF = HPC * HD

    inv_freq = 1.0 / (ROPE_BASE ** (np.arange(0, HD, 2, dtype=np.float64) / HD))
    pos = np.asarray(positions).astype(np.float64)[:S]
    freq = pos[None, :] * inv_freq[:, None]
    c = np.cos(freq).astype(np.float32)
    s = np.sin(freq).astype(np.float32)
    cosf = np.vstack([c, c]).astype(np.float16)
    sinf = np.vstack([-s, s]).astype(np.float16)

    dm = np.zeros((128, 4, 512), np.float32)
    for j in range(4):
        dm[:, j, 128 * j:128 * (j + 1)] = np.triu(np.ones((128, 128), np.float32))
        dm[:, j, 128 * (j + 1):] = 1.0
    dmask = dm.reshape(128, 4 * 512).astype(np.float16)

    ones8 = np.ones((128, 128), np.float16)
    ident = np.eye(128, dtype=np.float16)

    perm = np.concatenate([np.arange(0, HD, 2), np.arange(1, HD, 2)])

    in_maps = []
    for c_id in range(N_CORES):
        b, hh = c_id // 2, c_id % 2
        xT = np.ascontiguousarray(x[b, :S, :].T).astype(np.float16)  # [D, S]
        f0 = hh * QF
        Wq = Wqkv[:, f0:f0 + QF]
        Wk = Wqkv[:, D + f0:D + f0 + QF]
        Wv = Wqkv[:, 2 * D + f0:2 * D + f0 + QF]
        Wqp = Wq.reshape(D, HPC, HD)[:, :, perm]       # [D, h, o]
        Wkp = Wk.reshape(D, HPC, HD)[:, :, perm]
        Wvp = Wv.reshape(D, HPC, HD)

        def pack(Wn):
            # [D, h, o] -> [h][d%128][k][o]
            return np.ascontiguousarray(
                Wn.reshape(KT, 128, HPC, HD).transpose(2, 1, 0, 3)
            ).astype(np.float16)

        wo = np.ascontiguousarray(
            Wout[f0:f0 + QF, :].reshape(HPC, 128, D)).astype(np.float16)
        in_maps.append({
            "xt": xT.reshape(KT, 128, S),
            "wq": pack(Wqp), "wk": pack(Wkp), "wv": pack(Wvp),
            "wo": wo,
            "cosf": cosf, "sinf": sinf, "dmask": dmask, "ones": ones8,
            "ident": ident,
        })
    return in_maps


def kernel(x, positions, mask, Wqkv, Wout):
    x = np.asarray(x, dtype=np.float32)
    Wqkv = np.asarray(Wqkv, dtype=np.float32)
    Wout = np.asarray(Wout, dtype=np.float32)
    S = x.shape[1]
    nc = build_nc(S=S)
    in_maps = prep_in_maps(x, positions, Wqkv, Wout, S=S)
    res = run_bass_kernel_spmd(nc, in_maps, core_ids=list(range(N_CORES)))
    outs = [res.results[c]["out"] for c in range(N_CORES)]
    full = np.stack([outs[2 * b] + outs[2 * b + 1] for b in range(B)], axis=0)
    return full.astype(np.float32)
